# revision 2
# baseline (speedup 1.0000x reference)
"""Trainium2 Bass kernel for nn_BinarizeLayer (chain Viterbi binarization).

Algorithm (see kernel_v1_backup.py for the full derivation)
-----------------------------------------------------------
2-state Viterbi over an 8.4M chain collapses to a clamp recurrence
d_i = e_i + clamp(d_{i-1}, -lam, lam); conjugating by prefix sums
(SB = cumsum((1-2p)/(2lam)), w = d~ + SB + lam~) gives the tube walk

    w_k = min(max(w_{k-1}, SB_k), SB_k + 1)

and backtracking reduces to "label = type of the nearest decisive
position to the right", where with h = w - SB_incl:
  h > 1  -> label 1 regardless of the future  (decisive-1)
  h <= 0 -> label 0 regardless                (decisive-0)
  else   -> copy the label from the right.

Engine split (per core), v2 with runtime-registered custom DVE ops:
  * DVE  ANT_CHAIN_PREFIX:  SB = scan(ADD, x*C1 + C2, init=C0-chained)
         -- fuses the ebar affine into a 1-op scan (~1.07 ns/col vs
         2.15 for the stock 2-op scan; single-stage feedback).
  * DVE  stock tensor_tensor_scan: the w tube walk (max,min) -- the
         only genuinely 2-op recurrence, irreducible.
  * DVE  ANT_CHAIN_BWDLBL:  labels = [M1 > M0], M1/M0 = running max
         (reversed APs) of a monotone tag where h>1 / h<=0. One custom
         scan replaces the stock backward scan. Blocks are seeded by a
         32-col warm-up instead of chaining (decisive events occur
         every ~3 cols; same forgetting argument as the row halos).
  * ACT  SP = SB + 1 (the W scan's second operand stream).
  * Pool/GpSimd: h = w - SB_incl (one tensor_tensor), the descending
         iota ramp (tag stream), and SP for the first blocks (ACT's
         queue boots late).

Sharding: 8 cores x 128 rows x 8192 payload, 32-col halos both sides
(clamp recurrences forget their initial state; chain ends padded with
p=0.5). Final-label boundary injected by a +-1e38 sentinel in w at the
last halo column, which the decisive rule picks up as a forced label.
"""

import numpy as np

import concourse.bass as bass
import concourse.mybir as mybir
from concourse import tile
from concourse import bass_utils
from concourse import library_overlay

import concourse.dve_ops as _D
from concourse.dve_spec import (
    Spec, Src0, Src1, C0, C1, C2, Zero, One, MaxNeg,
    scan, select, lower as _dve_lower, AluOp, _has_src1,
)
from concourse.dve_uop import DveOpSpec

LAM = 0.75
N = 8388608
NCORES = 8
P = 128          # partitions
HW = 32          # halo / warm-up width
D = 8192         # payload elements per partition row
R = D + 2 * HW   # row length incl. halos

BWD_MODE = "custom"   # "custom" | "stock"

# forward blocks: small first so compute starts as soon as DMA lands
_FWD_W = [256, 512, 1024, 1536, 2048, 2048, 832]
assert sum(_FWD_W) == R
FWD_BLOCKS = []
_c = 0
for _w in _FWD_W:
    FWD_BLOCKS.append((_c, _w))
    _c += _w
N_GPS_SP = 3     # leading blocks whose SP runs on GpSimd (ACT boots late)

# backward blocks, left->right (payload cols [HW, HW+D)); each block also
# re-scans a 32-col warm-up strip on its right, overwritten by the next
# block's payload (or landing in the right halo for the last block)
_BWD_W = [2048, 2048, 2048, 1536, 512]
assert sum(_BWD_W) == D
BWD_BLOCKS = []
_c = HW
for _w in _BWD_W:
    BWD_BLOCKS.append((_c, _w))
    _c += _w
RAMP_W = max(_BWD_W) + HW

# stock-mode backward blocks (right-to-left, chained; baseline scheme)
_BWD_W_STOCK = [128, 1920, 2048, 2048, 2048]
assert sum(_BWD_W_STOCK) == D
BWD_BLOCKS_STOCK = []
_c = HW
for _w in _BWD_W_STOCK:
    BWD_BLOCKS_STOCK.append((_c, _w))
    _c += _w


def _register_op(name, spec, subdim=False):
    for o in _D.OPS:
        if o.name == name:
            return o
    row = _D._CUSTOM_DVE_ROW_BASE + len(_D.OPS)
    _D._SUB_OPCODE_FOR_NAME[name] = row
    ver = "v3"
    uops = _dve_lower(spec, ver=ver)
    sha = DveOpSpec(name=name, opcode=row, uops=uops,
                    rd1_en=_has_src1(spec)).sha(ver)
    op = _D.DveOp(name, spec, subdim=subdim, uops_sha={ver: sha})
    _D.OPS.append(op)
    _D.CUSTOM_DVE_SPECS[name] = spec
    return op


# out[k] = c0 + sum_{j<=k} (in0[j]*c1 + c2)
PREFIX_OP = _register_op(
    "ANT_CHAIN_PREFIX",
    Spec(
        body=scan(AluOp.ADD, Src0 * C1 + C2, init=C0),
        reference=lambda in0, in1, s0, s1, imm2: (
            np.cumsum(in0.astype(np.float32) * s1 + imm2, axis=-1,
                      dtype=np.float32) + s0),
    ),
)

# labels = [M1 > M0]; M1 = runmax(select(in0 > 1, in1, -FLT_MAX)),
# M0 = runmax(select(in0 <= 0, in1, -FLT_MAX)); in0 = h, in1 = tag ramp
BWDLBL_OP = _register_op(
    "ANT_CHAIN_BWDLBL",
    Spec(
        body=scan(AluOp.MAX, select(Src0 > One, Src1, MaxNeg))
        > scan(AluOp.MAX, select(Src0 <= Zero, Src1, MaxNeg)),
    ),
)


def _build():
    f32 = mybir.dt.float32
    i8 = mybir.dt.int8
    Alu = mybir.AluOpType
    Copy = mybir.ActivationFunctionType.Copy

    nc = bass.Bass()
    x = nc.dram_tensor("x", [P, R], f32, kind="ExternalInput")
    y = nc.dram_tensor("y", [P, D], i8, kind="ExternalOutput")

    with tile.TileContext(nc) as tc:
        with tc.tile_pool(name="big", bufs=1) as big:
            XT = big.tile([P, R], f32)        # input p, then w in place
            SB = big.tile([P, R + 1], f32)    # inclusive prefix; col0 = 0
            SP = big.tile([P, R + 1], f32)    # SB + 1
            WT = XT                           # tube walk overwrites input
            LB = big.tile([P, R], i8)         # labels
            TMP = big.tile([P, 1], f32)
            CB = big.tile([P, 1], f32)
            if BWD_MODE == "custom":
                H = big.tile([P, R], f32)     # h = w - SB_incl
                RAMP = big.tile([P, RAMP_W], f32)  # descending tag ramp
                # descending fp32 iota (exact for this range)
                nc.gpsimd.iota(RAMP[:, :], [[-1, RAMP_W]], base=RAMP_W,
                               channel_multiplier=0,
                               allow_small_or_imprecise_dtypes=True)

            nc.vector.memset(SB[:, 0:1], 0.0)
            nc.vector.memset(SP[:, 0:1], 1.0)

            # ---- forward: SB custom scan + SP + stock tube walk ----
            # DVE emission order staggers W one block behind SBE so the
            # DVE never waits on SP production.
            prev_w = None  # pending W block
            for bi, (c0, bw) in enumerate(FWD_BLOCKS):
                nc.sync.dma_start(XT[:, c0:c0 + bw], x[:, c0:c0 + bw])
                init = 0.0 if c0 == 0 else SB[:, c0:c0 + 1]
                nc.vector._custom_dve(
                    PREFIX_OP,
                    out=SB[:, c0 + 1:c0 + 1 + bw],
                    in0=XT[:, c0:c0 + bw],
                    s0=init, s1=-2.0 / (2 * LAM), imm2=1.0 / (2 * LAM))
                if bi < N_GPS_SP:
                    nc.gpsimd.tensor_scalar(SP[:, c0 + 1:c0 + 1 + bw],
                                            SB[:, c0 + 1:c0 + 1 + bw],
                                            1.0, None, Alu.add)
                else:
                    nc.scalar.activation(SP[:, c0 + 1:c0 + 1 + bw],
                                         SB[:, c0 + 1:c0 + 1 + bw],
                                         Copy, bias=1.0)
                if prev_w is not None:
                    pc0, pbw = prev_w
                    winit = 0.5 if pc0 == 0 else WT[:, pc0 - 1:pc0]
                    nc.vector.tensor_tensor_scan(
                        WT[:, pc0:pc0 + pbw], SB[:, pc0:pc0 + pbw],
                        SP[:, pc0:pc0 + pbw], winit, Alu.max, Alu.min)
                    if BWD_MODE == "custom":
                        nc.gpsimd.tensor_tensor(
                            H[:, pc0:pc0 + pbw], WT[:, pc0:pc0 + pbw],
                            SB[:, pc0 + 1:pc0 + 1 + pbw], Alu.subtract)
                prev_w = (c0, bw)
            pc0, pbw = prev_w
            winit = WT[:, pc0 - 1:pc0]
            # last W block, minus the sentinel column
            nc.vector.tensor_tensor_scan(
                WT[:, pc0:pc0 + pbw - 1], SB[:, pc0:pc0 + pbw - 1],
                SP[:, pc0:pc0 + pbw - 1], winit, Alu.max, Alu.min)
            nc.vector.tensor_tensor_scan(
                WT[:, R - 1:R], SB[:, R - 1:R],
                SP[:, R - 1:R], WT[:, R - 2:R - 1], Alu.max, Alu.min)

            # sentinel: w[R-1] := +-1e38 by the sign of d~ there
            nc.vector.tensor_scalar(TMP[:], SB[:, R:R + 1], 0.5, None,
                                    Alu.add)
            nc.vector.tensor_tensor(CB[:], WT[:, R - 1:R], TMP[:],
                                    Alu.is_gt)
            nc.vector.tensor_scalar(WT[:, R - 1:R], CB[:], 2e38, -1e38,
                                    Alu.mult, Alu.add)

            if BWD_MODE == "custom":
                # h for the tail (post-sentinel): last block's last col
                nc.gpsimd.tensor_tensor(
                    H[:, pc0:R], WT[:, pc0:R],
                    SB[:, pc0 + 1:R + 1], Alu.subtract)
                # ---- backward: independent blocks, 32-col warm-up ----
                for c0, bw in BWD_BLOCKS:
                    wd = bw + HW if c0 + bw + HW <= R else R - c0
                    nc.vector._custom_dve(
                        BWDLBL_OP,
                        out=LB[:, c0:c0 + wd][:, ::-1],
                        in0=H[:, c0:c0 + wd][:, ::-1],
                        in1=RAMP[:, 0:wd][:, ::-1])
                    nc.sync.dma_start(y[:, c0 - HW:c0 - HW + bw],
                                      LB[:, c0:c0 + bw])
            else:
                for si in range(len(BWD_BLOCKS_STOCK) - 1, -1, -1):
                    c0, bw = BWD_BLOCKS_STOCK[si]
                    last = si == len(BWD_BLOCKS_STOCK) - 1
                    wd = bw + (HW if last else 0)
                    init = 0.0 if last else LB[:, c0 + wd:c0 + wd + 1]
                    nc.vector.tensor_tensor_scan(
                        LB[:, c0:c0 + wd][:, ::-1],
                        WT[:, c0:c0 + wd][:, ::-1],
                        SP[:, c0 + 1:c0 + 1 + wd][:, ::-1],
                        init, Alu.add, Alu.is_gt)
                    nc.sync.dma_start(y[:, c0 - HW:c0 - HW + bw],
                                      LB[:, c0:c0 + bw])
    return nc


def _legalize_waits(nc, limit=1):
    """Split instructions carrying more than `limit` sem-waits (ISA cap)."""
    for fn in nc.m.functions:
        for blk in fn.blocks:
            insts = blk.instructions
            i = 0
            while i < len(insts):
                inst = insts[i]
                si = getattr(inst, "sync_info", None)
                if si is not None and si.on_wait and len(si.on_wait) > limit:
                    waits = list(si.on_wait)
                    inst.sync_info = mybir.SyncInfo(
                        on_wait=waits[-limit:], on_update=list(si.on_update))
                    pending = waits[:-limit]
                    for j in range(0, len(pending), limit):
                        nop = mybir.InstNoOp(
                            name=nc.get_next_instruction_name(),
                            sync_info=mybir.SyncInfo(
                                on_wait=pending[j:j + limit], on_update=[]),
                            bass_nofuse=True,
                            engine=inst.engine,
                        )
                        insts.insert(i, nop)
                        i += 1
                i += 1
    return nc


_nc_cache = None


def _get_nc():
    global _nc_cache
    if _nc_cache is None:
        nc = _legalize_waits(_build())
        library_overlay.lower_extended_insts(nc)
        _nc_cache = nc
    return _nc_cache


def _shard(inputs: np.ndarray):
    p = np.ascontiguousarray(inputs, dtype=np.float32)
    assert p.shape == (N,)
    pad = np.full(HW, 0.5, np.float32)
    pp = np.concatenate([pad, p, pad])
    nrows = N // D
    X = np.lib.stride_tricks.as_strided(pp, (nrows, R), (D * 4, 4))
    return [{"x": np.ascontiguousarray(X[k * P:(k + 1) * P])}
            for k in range(NCORES)]


def _run(inputs: np.ndarray, trace: bool = False):
    in_maps = _shard(inputs)
    res = bass_utils.run_bass_kernel_spmd(_get_nc(), in_maps,
                                          core_ids=list(range(NCORES)),
                                          trace=trace)
    lab = np.concatenate([np.asarray(res.results[k]["y"]).reshape(-1)
                          for k in range(NCORES)])
    return lab.astype(np.int32), res


def kernel(inputs: np.ndarray) -> np.ndarray:
    lab, _ = _run(inputs, trace=False)
    return lab


# revision 3
# speedup vs baseline: 1.3606x; 1.3606x over previous
"""Trainium2 Bass kernel for nn_BinarizeLayer (chain Viterbi binarization).

Algorithm (see kernel_v1_backup.py for the full derivation)
-----------------------------------------------------------
2-state Viterbi over an 8.4M chain collapses to a clamp recurrence
d_i = e_i + clamp(d_{i-1}, -lam, lam); conjugating by prefix sums
(SB = cumsum((1-2p)/(2lam)), w = d~ + SB + lam~) gives the tube walk

    w_k = min(max(w_{k-1}, SB_k), SB_k + 1)

and backtracking reduces to "label = type of the nearest decisive
position to the right", where with h = w - SB_incl:
  h > 1  -> label 1 regardless of the future  (decisive-1)
  h <= 0 -> label 0 regardless                (decisive-0)
  else   -> copy the label from the right.

Engine split (per core), v2 with runtime-registered custom DVE ops:
  * DVE  ANT_CHAIN_PREFIX:  SB = scan(ADD, x*C1 + C2, init=C0-chained)
         -- fuses the ebar affine into a 1-op scan (~1.07 ns/col vs
         2.15 for the stock 2-op scan; single-stage feedback).
  * DVE  stock tensor_tensor_scan: the w tube walk (max,min) -- the
         only genuinely 2-op recurrence, irreducible.
  * DVE  ANT_CHAIN_BWDLBL:  labels = [M1 > M0], M1/M0 = running max
         (reversed APs) of a monotone tag where h>1 / h<=0. One custom
         scan replaces the stock backward scan. Blocks are seeded by a
         32-col warm-up instead of chaining (decisive events occur
         every ~3 cols; same forgetting argument as the row halos).
  * ACT  SP = SB + 1 (the W scan's second operand stream).
  * Pool/GpSimd: h = w - SB_incl (one tensor_tensor), the descending
         iota ramp (tag stream), and SP for the first blocks (ACT's
         queue boots late).

Sharding: 8 cores x 128 rows x 8192 payload, 32-col halos both sides
(clamp recurrences forget their initial state; chain ends padded with
p=0.5). Final-label boundary injected by a +-1e38 sentinel in w at the
last halo column, which the decisive rule picks up as a forced label.
"""

import numpy as np

import concourse.bass as bass
import concourse.mybir as mybir
from concourse import tile
from concourse import bass_utils
from concourse import library_overlay

import concourse.dve_ops as _D
from concourse.dve_spec import (
    Spec, Src0, Src1, C0, C1, C2, Zero, One, MaxNeg,
    scan, select, lower as _dve_lower, AluOp, _has_src1,
)
from concourse.dve_uop import DveOpSpec

LAM = 0.75
N = 8388608
NCORES = 8
P = 128          # partitions
HW = 32          # halo / warm-up width
D = 8192         # payload elements per partition row
R = D + 2 * HW   # row length incl. halos

BWD_MODE = "custom"   # "custom" | "stock"

# forward blocks: small first so compute starts as soon as DMA lands
_FWD_W = [256, 512, 1024, 1536, 2048, 2048, 832]
assert sum(_FWD_W) == R
FWD_BLOCKS = []
_c = 0
for _w in _FWD_W:
    FWD_BLOCKS.append((_c, _w))
    _c += _w
N_GPS_SP = 0     # gpsimd tensor_scalar measured ~14 ns/col here — ACT only

# backward blocks, left->right (payload cols [HW, HW+D)); each block also
# re-scans a 32-col warm-up strip on its right, overwritten by the next
# block's payload (or landing in the right halo for the last block)
_BWD_W = [2048, 2048, 2048, 1536, 512]
assert sum(_BWD_W) == D
BWD_BLOCKS = []
_c = HW
for _w in _BWD_W:
    BWD_BLOCKS.append((_c, _w))
    _c += _w
RAMP_W = max(_BWD_W) + HW

# stock-mode backward blocks (right-to-left, chained; baseline scheme)
_BWD_W_STOCK = [128, 1920, 2048, 2048, 2048]
assert sum(_BWD_W_STOCK) == D
BWD_BLOCKS_STOCK = []
_c = HW
for _w in _BWD_W_STOCK:
    BWD_BLOCKS_STOCK.append((_c, _w))
    _c += _w


def _register_op(name, spec, subdim=False):
    for o in _D.OPS:
        if o.name == name:
            return o
    row = _D._CUSTOM_DVE_ROW_BASE + len(_D.OPS)
    _D._SUB_OPCODE_FOR_NAME[name] = row
    ver = "v3"
    uops = _dve_lower(spec, ver=ver)
    sha = DveOpSpec(name=name, opcode=row, uops=uops,
                    rd1_en=_has_src1(spec)).sha(ver)
    op = _D.DveOp(name, spec, subdim=subdim, uops_sha={ver: sha})
    _D.OPS.append(op)
    _D.CUSTOM_DVE_SPECS[name] = spec
    return op


# out[k] = c0 + sum_{j<=k} (in0[j]*c1 + c2)
PREFIX_OP = _register_op(
    "ANT_CHAIN_PREFIX",
    Spec(
        body=scan(AluOp.ADD, Src0 * C1 + C2, init=C0),
        reference=lambda in0, in1, s0, s1, imm2: (
            np.cumsum(in0.astype(np.float32) * s1 + imm2, axis=-1,
                      dtype=np.float32) + s0),
    ),
)

# labels = [M1 > M0]; M1 = runmax(select(in0 > 1, in1, -FLT_MAX)),
# M0 = runmax(select(in0 <= 0, in1, -FLT_MAX)); in0 = h, in1 = tag ramp
BWDLBL_OP = _register_op(
    "ANT_CHAIN_BWDLBL",
    Spec(
        body=scan(AluOp.MAX, select(Src0 > One, Src1, MaxNeg))
        > scan(AluOp.MAX, select(Src0 <= Zero, Src1, MaxNeg)),
    ),
)


def _build():
    f32 = mybir.dt.float32
    i8 = mybir.dt.int8
    Alu = mybir.AluOpType
    Copy = mybir.ActivationFunctionType.Copy

    nc = bass.Bass()
    x = nc.dram_tensor("x", [P, R], f32, kind="ExternalInput")
    y = nc.dram_tensor("y", [P, D], i8, kind="ExternalOutput")

    with tile.TileContext(nc) as tc:
        with tc.tile_pool(name="big", bufs=1) as big:
            XT = big.tile([P, R], f32)        # input p, then w in place
            SB = big.tile([P, R + 1], f32)    # inclusive prefix; col0 = 0
            SP = big.tile([P, R + 1], f32)    # SB + 1
            WT = XT                           # tube walk overwrites input
            LB = big.tile([P, R], i8)         # labels
            TMP = big.tile([P, 1], f32)
            CB = big.tile([P, 1], f32)
            if BWD_MODE == "custom":
                H = big.tile([P, R], f32)     # h = w - SB_incl
                RAMP = big.tile([P, RAMP_W], f32)  # descending tag ramp
                # descending fp32 iota (exact for this range)
                nc.gpsimd.iota(RAMP[:, :], [[-1, RAMP_W]], base=RAMP_W,
                               channel_multiplier=0,
                               allow_small_or_imprecise_dtypes=True)

            nc.vector.memset(SB[:, 0:1], 0.0)
            nc.vector.memset(SP[:, 0:1], 1.0)

            # ---- forward: SB custom scan + SP + stock tube walk ----
            # DVE emission order staggers W one block behind SBE so the
            # DVE never waits on SP production.
            prev_w = None  # pending W block
            for bi, (c0, bw) in enumerate(FWD_BLOCKS):
                nc.sync.dma_start(XT[:, c0:c0 + bw], x[:, c0:c0 + bw])
                init = 0.0 if c0 == 0 else SB[:, c0:c0 + 1]
                nc.vector._custom_dve(
                    PREFIX_OP,
                    out=SB[:, c0 + 1:c0 + 1 + bw],
                    in0=XT[:, c0:c0 + bw],
                    s0=init, s1=-2.0 / (2 * LAM), imm2=1.0 / (2 * LAM))
                if bi < N_GPS_SP:
                    nc.gpsimd.tensor_scalar(SP[:, c0 + 1:c0 + 1 + bw],
                                            SB[:, c0 + 1:c0 + 1 + bw],
                                            1.0, None, Alu.add)
                else:
                    nc.scalar.activation(SP[:, c0 + 1:c0 + 1 + bw],
                                         SB[:, c0 + 1:c0 + 1 + bw],
                                         Copy, bias=1.0)
                if prev_w is not None:
                    pc0, pbw = prev_w
                    winit = 0.5 if pc0 == 0 else WT[:, pc0 - 1:pc0]
                    nc.vector.tensor_tensor_scan(
                        WT[:, pc0:pc0 + pbw], SB[:, pc0:pc0 + pbw],
                        SP[:, pc0:pc0 + pbw], winit, Alu.max, Alu.min)
                    if BWD_MODE == "custom":
                        nc.gpsimd.tensor_tensor(
                            H[:, pc0:pc0 + pbw], WT[:, pc0:pc0 + pbw],
                            SB[:, pc0 + 1:pc0 + 1 + pbw], Alu.subtract)
                prev_w = (c0, bw)
            pc0, pbw = prev_w
            winit = WT[:, pc0 - 1:pc0]
            # last W block, minus the sentinel column
            nc.vector.tensor_tensor_scan(
                WT[:, pc0:pc0 + pbw - 1], SB[:, pc0:pc0 + pbw - 1],
                SP[:, pc0:pc0 + pbw - 1], winit, Alu.max, Alu.min)
            nc.vector.tensor_tensor_scan(
                WT[:, R - 1:R], SB[:, R - 1:R],
                SP[:, R - 1:R], WT[:, R - 2:R - 1], Alu.max, Alu.min)

            # sentinel: w[R-1] := +-1e38 by the sign of d~ there
            nc.vector.tensor_scalar(TMP[:], SB[:, R:R + 1], 0.5, None,
                                    Alu.add)
            nc.vector.tensor_tensor(CB[:], WT[:, R - 1:R], TMP[:],
                                    Alu.is_gt)
            nc.vector.tensor_scalar(WT[:, R - 1:R], CB[:], 2e38, -1e38,
                                    Alu.mult, Alu.add)

            if BWD_MODE == "custom":
                # h for the tail (post-sentinel): last block's last col
                nc.gpsimd.tensor_tensor(
                    H[:, pc0:R], WT[:, pc0:R],
                    SB[:, pc0 + 1:R + 1], Alu.subtract)
                # ---- backward: independent blocks, 32-col warm-up ----
                for c0, bw in BWD_BLOCKS:
                    wd = bw + HW if c0 + bw + HW <= R else R - c0
                    nc.vector._custom_dve(
                        BWDLBL_OP,
                        out=LB[:, c0:c0 + wd][:, ::-1],
                        in0=H[:, c0:c0 + wd][:, ::-1],
                        in1=RAMP[:, 0:wd][:, ::-1])
                    nc.sync.dma_start(y[:, c0 - HW:c0 - HW + bw],
                                      LB[:, c0:c0 + bw])
            else:
                for si in range(len(BWD_BLOCKS_STOCK) - 1, -1, -1):
                    c0, bw = BWD_BLOCKS_STOCK[si]
                    last = si == len(BWD_BLOCKS_STOCK) - 1
                    wd = bw + (HW if last else 0)
                    init = 0.0 if last else LB[:, c0 + wd:c0 + wd + 1]
                    nc.vector.tensor_tensor_scan(
                        LB[:, c0:c0 + wd][:, ::-1],
                        WT[:, c0:c0 + wd][:, ::-1],
                        SP[:, c0 + 1:c0 + 1 + wd][:, ::-1],
                        init, Alu.add, Alu.is_gt)
                    nc.sync.dma_start(y[:, c0 - HW:c0 - HW + bw],
                                      LB[:, c0:c0 + bw])
    return nc


def _legalize_waits(nc, limit=1):
    """Split instructions carrying more than `limit` sem-waits (ISA cap)."""
    for fn in nc.m.functions:
        for blk in fn.blocks:
            insts = blk.instructions
            i = 0
            while i < len(insts):
                inst = insts[i]
                si = getattr(inst, "sync_info", None)
                if si is not None and si.on_wait and len(si.on_wait) > limit:
                    waits = list(si.on_wait)
                    inst.sync_info = mybir.SyncInfo(
                        on_wait=waits[-limit:], on_update=list(si.on_update))
                    pending = waits[:-limit]
                    for j in range(0, len(pending), limit):
                        nop = mybir.InstNoOp(
                            name=nc.get_next_instruction_name(),
                            sync_info=mybir.SyncInfo(
                                on_wait=pending[j:j + limit], on_update=[]),
                            bass_nofuse=True,
                            engine=inst.engine,
                        )
                        insts.insert(i, nop)
                        i += 1
                i += 1
    return nc


_nc_cache = None


def _get_nc():
    global _nc_cache
    if _nc_cache is None:
        nc = _legalize_waits(_build())
        library_overlay.lower_extended_insts(nc)
        _nc_cache = nc
    return _nc_cache


def _shard(inputs: np.ndarray):
    p = np.ascontiguousarray(inputs, dtype=np.float32)
    assert p.shape == (N,)
    pad = np.full(HW, 0.5, np.float32)
    pp = np.concatenate([pad, p, pad])
    nrows = N // D
    X = np.lib.stride_tricks.as_strided(pp, (nrows, R), (D * 4, 4))
    return [{"x": np.ascontiguousarray(X[k * P:(k + 1) * P])}
            for k in range(NCORES)]


def _run(inputs: np.ndarray, trace: bool = False):
    in_maps = _shard(inputs)
    res = bass_utils.run_bass_kernel_spmd(_get_nc(), in_maps,
                                          core_ids=list(range(NCORES)),
                                          trace=trace)
    lab = np.concatenate([np.asarray(res.results[k]["y"]).reshape(-1)
                          for k in range(NCORES)])
    return lab.astype(np.int32), res


def kernel(inputs: np.ndarray) -> np.ndarray:
    lab, _ = _run(inputs, trace=False)
    return lab
